# revision 4
# baseline (speedup 1.0000x reference)
"""Bass/Trainium2 kernel for nn_BivariateSpectral: batched smallest-eigenvalue of
S_b = sym(A + B*diag(x_b) + C*diag(y_b)), b = 0..32767, each 64x64, 8 NeuronCores.

Algorithm (per core, data-parallel over batch):
  Phase 1 - batched Lanczos (K steps) on D_b = (M_b + M_b^T)/64 = S_b/32.
    Batched matvec via shared 128x128 block-diag matmuls (dim on partitions,
    two batch-halves packed on partition halves, batch on the free dim):
      D v = Ah v + Bh (x*v) + Ch (y*v) + x*(Bh^T v) + y*(Ch^T v)
    Engine split per step: gpsimd does the front Hadamards (t1,t2,t4); PE does
    the 5 matvec passes, identity-matmul PSUM accumulation of the post-scaled
    transpose terms and the orthogonalization terms, and the ones-block-diag
    reduction broadcasts (alpha, beta^2); DVE does the remaining Hadamards
    (m12, p_t, t3) and the normalization divide; ACT does Square (beta^2 prep)
    and Sqrt (beta) from one activation table.  Tridiagonal rows (alpha,
    beta^2) are DMA'd straight from PSUM into SBUF staging tiles.
  Phase 2 - divide-form Sturm bisection (count of negative LDL^T pivots) on
    the K x K tridiagonals, batch on partitions, NS shifts/pass, arithmetic
    bracket update (no predicated copies).  Output scaled back by 32.
"""

import functools
import numpy as np

BATCH, DIM = 32768, 64
NCORES = 8
SHARD = BATCH // NCORES      # 4096 batch elems per core
NFREE = SHARD // 2           # 2048 free columns (two partition-halves)
CHUNK = 512                  # columns per group
NCH = NFREE // CHUNK         # 2 groups
K = 23                       # Lanczos steps
NPOW = 5                     # power-iteration boost steps for v0 (v0 <- D^NPOW v0)
ROWS_A = 2 * K               # 64 rows in alpha staging (2j+h)
ROWS_B = 2 * (K - 1)         # 62 rows in beta^2 staging
TG = NFREE // 128            # 16 transpose column-groups
NS = 8                       # bisection shifts per pass
PASSES = 3
C_OP = np.float32(1.0 / 64.0)   # A,B,C host prescale: D = (M+M^T)/64 = S/32
OUT_SCALE = 16.0                # lam_S = 32 * 0.5 * (lo+hi)


def _v0_vec():
    rng = np.random.default_rng(15)
    v = rng.standard_normal(DIM).astype(np.float64)
    v /= np.sqrt((v * v).sum())
    return v.astype(np.float32)


def _bd(m):
    """128x128 block-diagonal duplication of a 64x64 matrix."""
    out = np.zeros((128, 128), np.float32)
    out[:64, :64] = m
    out[64:, 64:] = m
    return out


def _bcast_s(ap, extra_off=0, count=2, ns=NS):
    """Insert a 0-step 'shift' dim after the partition dim of a [128, T, R] AP,
    slicing 'count' elems at free offset extra_off: -> [128, ns, T, count]."""
    import concourse.bass as bass
    dims = list(ap.ap)
    part = dims[0]
    tdim = dims[1]
    return bass.AP(
        tensor=ap.tensor,
        offset=ap.offset + extra_off,
        ap=[part, [0, ns], tdim, [1, count]],
    )


def _bcast_flat(ap, ns=NS):
    """[128, T, 2] AP -> [128, ns, T, 2] via 0-step shift dim."""
    import concourse.bass as bass
    dims = list(ap.ap)
    return bass.AP(tensor=ap.tensor, offset=ap.offset, ap=[dims[0], [0, ns]] + dims[1:])


def _rows2(ap, row0=0, row_step=64, cols=CHUNK):
    """AP for rows {row0, row0+row_step} x [0, cols) of a [128, C] tile."""
    import concourse.bass as bass
    dims = list(ap.ap)
    part = dims[0]
    pstride = part[0]
    return bass.AP(
        tensor=ap.tensor,
        offset=ap.offset,
        ap=[[pstride * row_step, 2], [1, cols]],
    )


def _perm_ns_last(ap, ns=NS, tg=TG):
    """[128, NS, TG, 2] AP -> [128, TG, 2, NS] (reduce target)."""
    import concourse.bass as bass
    dims = list(ap.ap)
    part, nsd, tgd, hd = dims
    return bass.AP(tensor=ap.tensor, offset=ap.offset, ap=[part, tgd, hd, nsd])


@functools.lru_cache(maxsize=4)
def _program(idx: int):
    import concourse.bacc as bacc
    import concourse.bass as bass
    import concourse.mybir as mybir
    import concourse.tile as tile
    from concourse.masks import make_identity

    F32 = mybir.dt.float32
    F32R = mybir.dt.float32r
    OP = mybir.AluOpType
    ACTF = mybir.ActivationFunctionType

    nc = bacc.Bacc("TRN2", target_bir_lowering=False, debug=False)

    xy_in = nc.dram_tensor("xy", [128, 2 * NFREE], F32, kind="ExternalInput").ap()
    lms_in = nc.dram_tensor("lms", [128, 128], F32, kind="ExternalInput").ap()
    lbf_in = nc.dram_tensor("lbf", [128, 128], F32, kind="ExternalInput").ap()
    lcf_in = nc.dram_tensor("lcf", [128, 128], F32, kind="ExternalInput").ap()
    lbt_in = nc.dram_tensor("lbt", [128, 128], F32, kind="ExternalInput").ap()
    lct_in = nc.dram_tensor("lct", [128, 128], F32, kind="ExternalInput").ap()
    obd_in = nc.dram_tensor("obd", [128, 128], F32, kind="ExternalInput").ap()
    idp_in = nc.dram_tensor("idp", [128, 128], F32, kind="ExternalInput").ap()
    idn_in = nc.dram_tensor("idn", [128, 128], F32, kind="ExternalInput").ap()
    v0_in = nc.dram_tensor("v0", [128, 1], F32, kind="ExternalInput").ap()
    lam_out = nc.dram_tensor("lam", [SHARD], F32, kind="ExternalOutput").ap()

    with tile.TileContext(nc) as tc:
        with tc.tile_pool(name="stage", bufs=1) as stg:
            ta_sb = stg.tile([ROWS_A, NFREE], F32)
            tb_sb = stg.tile([ROWS_B, NFREE], F32)

            # ---------------- Phase 1: Lanczos ----------------
            with (
                tc.tile_pool(name="singles", bufs=1) as singles,
                tc.tile_pool(name="vpool", bufs=3) as vpool,
                tc.tile_pool(name="bbp", bufs=2) as bbp,
                tc.tile_pool(name="wk1", bufs=2) as wk1,
                tc.tile_pool(name="wk2", bufs=1) as wk2,
                tc.tile_pool(name="prp", bufs=4) as prp,
                tc.tile_pool(name="ppw", bufs=1, space="PSUM") as ppw,
                tc.tile_pool(name="pps", bufs=1, space="PSUM") as pps,
            ):
                xyt = singles.tile([128, 2 * NFREE], F32)
                nc.sync.dma_start(out=xyt[:], in_=xy_in)
                stat_f32 = {}
                stat_r = {}
                for nm, src in (("lms", lms_in), ("lbf", lbf_in), ("lcf", lcf_in),
                                ("lbt", lbt_in), ("lct", lct_in), ("obd", obd_in),
                                ("idp", idp_in), ("idn", idn_in)):
                    t_f = singles.tile([128, 128], F32, tag=f"stf_{nm}",
                                       name=f"stf_{nm}")
                    nc.sync.dma_start(out=t_f[:], in_=src)
                    t_r = singles.tile([128, 128], F32R, tag=f"str_{nm}",
                                       name=f"str_{nm}")
                    nc.vector.tensor_copy(t_r[:], t_f[:])
                    stat_f32[nm] = t_f
                    stat_r[nm] = t_r
                lms_r, lbf_r, lcf_r = stat_r["lms"], stat_r["lbf"], stat_r["lcf"]
                lbt_r, lct_r, obd_r = stat_r["lbt"], stat_r["lct"], stat_r["obd"]
                idp_r, idn_r = stat_r["idp"], stat_r["idn"]
                v0t = singles.tile([128, 1], F32)
                nc.sync.dma_start(out=v0t[:], in_=v0_in)
                epst = singles.tile([128, 1], F32)
                nc.vector.memset(epst[:], 1e-12)

                st = []
                for g in range(NCH):
                    v_cur = vpool.tile([128, CHUNK], F32R, tag=f"v{g}")
                    nc.vector.tensor_copy(v_cur[:],
                                          v0t[:, 0:1].to_broadcast((128, CHUNK)))
                    st.append({"v": v_cur, "vp": None, "bb": None})

                # ---- power boost: v <- D^NPOW v0 (unnormalized), then normalize
                for it in range(NPOW):
                    PT = [{} for _ in range(NCH)]
                    for g in range(NCH):
                        D = PT[g]
                        xg = xyt[:, g * 2 * CHUNK: g * 2 * CHUNK + CHUNK]
                        yg = xyt[:, g * 2 * CHUNK + CHUNK: (g + 1) * 2 * CHUNK]
                        D["t1"] = wk1.tile([128, CHUNK], F32R, tag=f"t1{g}", name=f"pw_t1{g}")
                        D["t2"] = wk1.tile([128, CHUNK], F32R, tag=f"t2{g}", name=f"pw_t2{g}")
                        nc.gpsimd.tensor_mul(D["t1"][:], xg, st[g]["v"][:])
                        nc.vector.tensor_mul(D["t2"][:], yg, st[g]["v"][:])
                    for g in range(NCH):
                        PT[g]["p3"] = pps.tile([128, CHUNK], F32, tag=f"ps{g}", name=f"pw_p3{g}")
                        PT[g]["pw"] = ppw.tile([128, CHUNK], F32, tag=f"pw{g}", name=f"pw_pw{g}")
                        nc.tensor.matmul(PT[g]["p3"][:], lbt_r[:], st[g]["v"][:],
                                         start=True, stop=True)
                    for g in range(NCH):
                        D = PT[g]
                        D["m1"] = wk2.tile([128, CHUNK], F32R, tag=f"m1{g}", name=f"pw_m1{g}")
                        xg = xyt[:, g * 2 * CHUNK: g * 2 * CHUNK + CHUNK]
                        nc.vector.tensor_mul(D["m1"][:], xg, D["p3"][:])
                    for g in range(NCH):
                        nc.tensor.matmul(PT[g]["pw"][:], lms_r[:], st[g]["v"][:],
                                         start=True, stop=False)
                    for g in range(NCH):
                        nc.tensor.matmul(PT[g]["pw"][:], lbf_r[:], PT[g]["t1"][:],
                                         start=False, stop=False)
                    for g in range(NCH):
                        PT[g]["p4"] = pps.tile([128, CHUNK], F32, tag=f"ps{g}", name=f"pw_p4{g}")
                        nc.tensor.matmul(PT[g]["p4"][:], lct_r[:], st[g]["v"][:],
                                         start=True, stop=True)
                    for g in range(NCH):
                        nc.tensor.matmul(PT[g]["pw"][:], lcf_r[:], PT[g]["t2"][:],
                                         start=False, stop=False)
                    for g in range(NCH):
                        D = PT[g]
                        D["m2"] = wk2.tile([128, CHUNK], F32R, tag=f"m2{g}", name=f"pw_m2{g}")
                        yg = xyt[:, g * 2 * CHUNK + CHUNK: (g + 1) * 2 * CHUNK]
                        nc.vector.tensor_mul(D["m2"][:], yg, D["p4"][:])
                    for g in range(NCH):
                        nc.tensor.matmul(PT[g]["pw"][:], idp_r[:], PT[g]["m1"][:],
                                         start=False, stop=False)
                    for g in range(NCH):
                        nc.tensor.matmul(PT[g]["pw"][:], idp_r[:], PT[g]["m2"][:],
                                         start=False, stop=True)
                    for g in range(NCH):
                        v_nxt = vpool.tile([128, CHUNK], F32R, tag=f"v{g}")
                        nc.scalar.activation(v_nxt[:], PT[g]["pw"][:], ACTF.Copy)
                        st[g]["v"] = v_nxt
                # normalize the boosted v
                NT = [{} for _ in range(NCH)]
                for g in range(NCH):
                    D = NT[g]
                    D["sq"] = wk2.tile([128, CHUNK], F32R, tag=f"q{g}", name=f"nz_sq{g}")
                    nc.scalar.activation(D["sq"][:], st[g]["v"][:], ACTF.Square)
                for g in range(NCH):
                    NT[g]["bc"] = pps.tile([128, CHUNK], F32, tag=f"ps{g}", name=f"nz_bc{g}")
                    nc.tensor.matmul(NT[g]["bc"][:], obd_r[:], NT[g]["sq"][:],
                                     start=True, stop=True)
                for g in range(NCH):
                    D = NT[g]
                    D["nrm"] = bbp.tile([128, CHUNK], F32, tag=f"bb{g}", name=f"nz_nrm{g}")
                    nc.scalar.activation(D["nrm"][:], D["bc"][:], ACTF.Sqrt,
                                         bias=epst[:], scale=1.0)
                    D["rb"] = wk2.tile([128, CHUNK], F32, tag=f"rb{g}", name=f"nz_rb{g}")
                    nc.vector.reciprocal_approx_fast(out=D["rb"][:], in_=D["nrm"][:])
                    v_nxt = vpool.tile([128, CHUNK], F32R, tag=f"v{g}")
                    nc.gpsimd.tensor_mul(v_nxt[:], st[g]["v"][:], D["rb"][:])
                    st[g]["v"] = v_nxt

                for j in range(K):
                    last = j == K - 1
                    T = [{} for _ in range(NCH)]
                    # ---- gpsimd front Hadamards ----
                    for g in range(NCH):
                        S, D = st[g], T[g]
                        xg = xyt[:, g * 2 * CHUNK: g * 2 * CHUNK + CHUNK]
                        yg = xyt[:, g * 2 * CHUNK + CHUNK: (g + 1) * 2 * CHUNK]
                        D["t1"] = wk1.tile([128, CHUNK], F32R, tag=f"t1{g}", name=f"t1{g}")
                        D["t2"] = wk1.tile([128, CHUNK], F32R, tag=f"t2{g}", name=f"t2{g}")
                        nc.gpsimd.tensor_mul(D["t1"][:], xg, S["v"][:])
                        nc.gpsimd.tensor_mul(D["t2"][:], yg, S["v"][:])
                        if j > 0 and not last:
                            D["t4"] = wk2.tile([128, CHUNK], F32R, tag=f"t4{g}", name=f"t4{g}")
                            nc.gpsimd.tensor_mul(D["t4"][:], S["bb"][:], S["vp"][:])
                    # ---- PE matvec (stationary-grouped for LDWEIGHTS reuse) ----
                    for g in range(NCH):
                        T[g]["p3"] = pps.tile([128, CHUNK], F32, tag=f"ps{g}", name=f"p3{g}")
                        T[g]["pw"] = ppw.tile([128, CHUNK], F32, tag=f"pw{g}", name=f"pw{g}")
                    for g in range(NCH):
                        for n0 in range(0, CHUNK, 512):
                            ns = slice(n0, n0 + 512)
                            nc.tensor.matmul(T[g]["p3"][:, ns], lbt_r[:],
                                             st[g]["v"][:, ns], start=True, stop=True)
                    # m1 = x * (Bh^T v) frees the p3 slot for p4
                    for g in range(NCH):
                        D = T[g]
                        D["m1"] = wk2.tile([128, CHUNK], F32R, tag=f"m1{g}", name=f"m1{g}")
                        xg = xyt[:, g * 2 * CHUNK: g * 2 * CHUNK + CHUNK]
                        nc.vector.tensor_mul(D["m1"][:], xg, D["p3"][:])
                    # pw passes (independent) hide the p3->p4 slot turnaround
                    for g in range(NCH):
                        for n0 in range(0, CHUNK, 512):
                            ns = slice(n0, n0 + 512)
                            nc.tensor.matmul(T[g]["pw"][:, ns], lms_r[:],
                                             st[g]["v"][:, ns], start=True, stop=False)
                    for g in range(NCH):
                        for n0 in range(0, CHUNK, 512):
                            ns = slice(n0, n0 + 512)
                            nc.tensor.matmul(T[g]["pw"][:, ns], lbf_r[:],
                                             T[g]["t1"][:, ns], start=False, stop=False)
                    for g in range(NCH):
                        T[g]["p4"] = pps.tile([128, CHUNK], F32, tag=f"ps{g}", name=f"p4{g}")
                        for n0 in range(0, CHUNK, 512):
                            ns = slice(n0, n0 + 512)
                            nc.tensor.matmul(T[g]["p4"][:, ns], lct_r[:],
                                             st[g]["v"][:, ns], start=True, stop=True)
                    for g in range(NCH):
                        for n0 in range(0, CHUNK, 512):
                            ns = slice(n0, n0 + 512)
                            nc.tensor.matmul(T[g]["pw"][:, ns], lcf_r[:],
                                             T[g]["t2"][:, ns], start=False, stop=False)
                    for g in range(NCH):
                        D = T[g]
                        D["m2"] = wk2.tile([128, CHUNK], F32R, tag=f"m2{g}", name=f"m2{g}")
                        yg = xyt[:, g * 2 * CHUNK + CHUNK: (g + 1) * 2 * CHUNK]
                        nc.vector.tensor_mul(D["m2"][:], yg, D["p4"][:])
                    # ---- PE identity accumulate of m1, m2 into pw ----
                    for g in range(NCH):
                        for n0 in range(0, CHUNK, 512):
                            ns = slice(n0, n0 + 512)
                            nc.tensor.matmul(T[g]["pw"][:, ns], idp_r[:],
                                             T[g]["m1"][:, ns], start=False,
                                             stop=False)
                    for g in range(NCH):
                        for n0 in range(0, CHUNK, 512):
                            ns = slice(n0, n0 + 512)
                            nc.tensor.matmul(T[g]["pw"][:, ns], idp_r[:],
                                             T[g]["m2"][:, ns], start=False,
                                             stop=True)
                    # ---- DVE p_t = v * w ----
                    for g in range(NCH):
                        D = T[g]
                        D["pt"] = wk2.tile([128, CHUNK], F32R, tag=f"pt{g}", name=f"ptl{g}")
                        nc.vector.tensor_mul(D["pt"][:], st[g]["v"][:], D["pw"][:])
                    # ---- PE alpha broadcast ----
                    for g in range(NCH):
                        T[g]["bc"] = pps.tile([128, CHUNK], F32, tag=f"ps{g}", name=f"bc{g}")
                        for n0 in range(0, CHUNK, 512):
                            ns = slice(n0, n0 + 512)
                            nc.tensor.matmul(T[g]["bc"][:, ns], obd_r[:],
                                             T[g]["pt"][:, ns], start=True, stop=True)
                    # ---- stage alpha rows: ACT bounce (aligned) + DMA to row 2j ----
                    for g in range(NCH):
                        csl = slice(g * CHUNK, (g + 1) * CHUNK)
                        pr = prp.tile([2, CHUNK], F32, tag="pr", name=f"pr{g}")
                        nc.scalar.activation(pr[:], T[g]["bc"][0:2, 0:CHUNK],
                                             ACTF.Copy)
                        nc.sync.dma_start(out=ta_sb[2 * j: 2 * j + 2, csl],
                                          in_=pr[:])
                    if last:
                        continue
                    # ---- DVE t3 = alpha * v ----
                    for g in range(NCH):
                        D = T[g]
                        D["t3"] = wk2.tile([128, CHUNK], F32R, tag=f"t3{g}", name=f"t3{g}")
                        nc.vector.tensor_mul(D["t3"][:], D["bc"][:], st[g]["v"][:])
                    # ---- PE -I accumulate of t3 (and t4) into pw ----
                    for g in range(NCH):
                        for n0 in range(0, CHUNK, 512):
                            ns = slice(n0, n0 + 512)
                            nc.tensor.matmul(T[g]["pw"][:, ns], idn_r[:],
                                             T[g]["t3"][:, ns], start=False,
                                             stop=(j == 0), skip_group_check=True)
                    if j > 0:
                        for g in range(NCH):
                            for n0 in range(0, CHUNK, 512):
                                ns = slice(n0, n0 + 512)
                                nc.tensor.matmul(T[g]["pw"][:, ns], idn_r[:],
                                                 T[g]["t4"][:, ns], start=False,
                                                 stop=True, skip_group_check=True)
                    # ---- ACT ws = copy(w_hat) to SBUF; q = Square(ws) ----
                    for g in range(NCH):
                        D = T[g]
                        D["ws"] = wk2.tile([128, CHUNK], F32R, tag=f"ws{g}", name=f"ws{g}")
                        nc.scalar.activation(D["ws"][:], D["pw"][:], ACTF.Copy)
                    for g in range(NCH):
                        D = T[g]
                        D["q"] = wk2.tile([128, CHUNK], F32R, tag=f"q{g}", name=f"q{g}")
                        nc.scalar.activation(D["q"][:], D["ws"][:], ACTF.Square)
                    # ---- PE beta^2 broadcast (reuse bc tag -> WAR dep) ----
                    for g in range(NCH):
                        T[g]["bc2"] = pps.tile([128, CHUNK], F32, tag=f"ps{g}", name=f"bc2{g}")
                        for n0 in range(0, CHUNK, 512):
                            ns = slice(n0, n0 + 512)
                            nc.tensor.matmul(T[g]["bc2"][:, ns], obd_r[:],
                                             T[g]["q"][:, ns], start=True, stop=True)
                    # ---- ACT bb = Sqrt(beta^2 + eps); v_nxt = ws * (1/bb) ----
                    # beta rows staged straight from bb (SBUF) via DMA;
                    # phase 2 squares them back to beta^2.
                    for g in range(NCH):
                        D = T[g]
                        bbn = bbp.tile([128, CHUNK], F32, tag=f"bb{g}")
                        nc.scalar.activation(bbn[:], D["bc2"][:], ACTF.Sqrt,
                                             bias=epst[:], scale=1.0)
                        csl = slice(g * CHUNK, (g + 1) * CHUNK)
                        nc.sync.dma_start(out=tb_sb[2 * j: 2 * j + 2, csl],
                                          in_=bbn[0:2, 0:CHUNK])
                        rb = wk2.tile([128, CHUNK], F32, tag=f"rb{g}", name=f"rb{g}")
                        nc.vector.reciprocal_approx_fast(out=rb[:], in_=bbn[:])
                        v_nxt = vpool.tile([128, CHUNK], F32R, tag=f"v{g}")
                        nc.gpsimd.tensor_mul(v_nxt[:], D["ws"][:], rb[:])
                        st[g]["vp"] = st[g]["v"]
                        st[g]["v"] = v_nxt
                        st[g]["bb"] = bbn

            # ---------------- Phase 2: transpose + Sturm bisection ----------------
            with (
                tc.tile_pool(name="bis", bufs=1) as bis,
                tc.tile_pool(name="st3", bufs=1) as st3,
                tc.tile_pool(name="pt", bufs=2, space="PSUM") as ptp,
            ):
                ident = bis.tile([128, 128], F32)
                make_identity(nc, ident[:])

                td_a = bis.tile([128, TG, ROWS_A], F32)
                td_b = bis.tile([128, TG, ROWS_B], F32)
                for t in range(TG):
                    csl = slice(t * 128, (t + 1) * 128)
                    pa = ptp.tile([128, ROWS_A], F32, tag="pt")
                    nc.tensor.transpose(pa[:], ta_sb[:, csl],
                                        ident[0:ROWS_A, 0:ROWS_A])
                    nc.vector.tensor_copy(td_a[:, t, :], pa[:])
                    pb = ptp.tile([128, ROWS_B], F32, tag="pt")
                    nc.tensor.transpose(pb[:], tb_sb[:, csl],
                                        ident[0:ROWS_B, 0:ROWS_B])
                    nc.vector.tensor_copy(td_b[:, t, :], pb[:])


                import concourse.bass as bass_mod

                def jdims_ap(tile_ap, nj, step0=2):
                    d = list(tile_ap.ap)
                    return bass_mod.AP(
                        tensor=tile_ap.tensor, offset=tile_ap.offset,
                        ap=[d[0], d[1], [1, 2], [step0, nj]],
                    )

                # td_b holds |beta_j| (= sqrt(beta^2+eps)) as staged
                absb = td_b
                g_t = bis.tile([128, TG, ROWS_A], F32)
                nc.vector.tensor_copy(g_t[:], td_a[:])
                nc.vector.tensor_sub(g_t[:, :, 2:ROWS_A], g_t[:, :, 2:ROWS_A], absb[:])
                nc.vector.tensor_sub(g_t[:, :, 0:ROWS_B], g_t[:, :, 0:ROWS_B], absb[:])

                lo = bis.tile([128, TG, 2], F32)
                hi = bis.tile([128, TG, 2], F32)
                nc.vector.tensor_reduce(lo[:], jdims_ap(g_t[:], K),
                                        mybir.AxisListType.X, OP.min)
                if idx == 0:
                    nc.vector.tensor_reduce(hi[:], jdims_ap(td_a[:], K),
                                            mybir.AxisListType.X, OP.min)
                else:
                    g2 = g_t
                    nc.vector.tensor_copy(g2[:], td_a[:])
                    nc.vector.tensor_add(g2[:, :, 2:ROWS_A], g2[:, :, 2:ROWS_A],
                                         absb[:])
                    nc.vector.tensor_add(g2[:, :, 0:ROWS_B], g2[:, :, 0:ROWS_B],
                                         absb[:])
                    nc.vector.tensor_reduce(hi[:], jdims_ap(g2[:], K),
                                            mybir.AxisListType.X, OP.max)

                # square |beta| back to beta^2 for the Sturm recurrence
                nc.vector.tensor_mul(td_b[:], td_b[:], td_b[:])

                cs = bis.tile([128, NS, TG, 2], F32)
                for s in range(NS):
                    nc.vector.memset(cs[:, s, :, :], float(s + 1) / float(NS + 1))

                sig = st3.tile([128, NS, TG, 2], F32, tag="sig")
                pA = st3.tile([128, NS, TG, 2], F32, tag="pA")
                pB = st3.tile([128, NS, TG, 2], F32, tag="pB")
                pC = st3.tile([128, NS, TG, 2], F32, tag="pC")
                ca_t = st3.tile([128, NS, TG, 2], F32, tag="ca")
                u_t = st3.tile([128, NS, TG, 2], F32, tag="u")
                tb_t = st3.tile([128, NS, TG, 2], F32, tag="tb")
                sg_t = st3.tile([128, NS, TG, 2], F32, tag="sg")
                cA = st3.tile([128, NS, TG, 2], F32, tag="cA")
                cB = st3.tile([128, NS, TG, 2], F32, tag="cB")
                les = st3.tile([128, NS, TG, 2], F32, tag="les")
                d_t = bis.tile([128, TG, 2], F32)
                m_t = bis.tile([128, TG, 2], F32)

                thr = float(idx) + 0.5
                for ip in range(PASSES):
                    nc.vector.tensor_sub(d_t[:], hi[:], lo[:])
                    nc.vector.tensor_mul(sig[:], cs[:], _bcast_flat(d_t[:]))
                    nc.vector.tensor_add(sig[:], sig[:], _bcast_flat(lo[:]))
                    po, pp, pn = pA, pB, pC
                    nc.vector.memset(po[:], 1.0)
                    nc.vector.tensor_sub(pp[:], _bcast_s(td_a[:], 0), sig[:])
                    cnt, cnt_nxt = cA, cB
                    nc.vector.tensor_scalar(out=cnt[:], in0=pp[:], scalar1=0.0,
                                            scalar2=None, op0=OP.is_lt)
                    for j in range(1, K):
                        nc.vector.tensor_sub(ca_t[:], _bcast_s(td_a[:], 2 * j),
                                             sig[:])
                        nc.vector.tensor_mul(u_t[:], ca_t[:], pp[:])
                        nc.vector.tensor_mul(tb_t[:], _bcast_s(td_b[:], 2 * (j - 1)),
                                             po[:])
                        nc.vector.tensor_sub(pn[:], u_t[:], tb_t[:])
                        nc.vector.tensor_mul(sg_t[:], pn[:], pp[:])
                        nc.vector.scalar_tensor_tensor(
                            out=cnt_nxt[:], in0=sg_t[:], scalar=0.0, in1=cnt[:],
                            op0=OP.is_lt, op1=OP.add)
                        po, pp, pn = pp, pn, po
                        cnt, cnt_nxt = cnt_nxt, cnt
                    # arithmetic bracket update: m = #shifts with cnt <= idx
                    nc.vector.tensor_scalar(out=les[:], in0=cnt[:], scalar1=thr,
                                            scalar2=None, op0=OP.is_le)
                    nc.vector.tensor_reduce(m_t[:], _perm_ns_last(les[:]),
                                            mybir.AxisListType.X, OP.add)
                    nc.vector.tensor_scalar(out=d_t[:], in0=d_t[:],
                                            scalar1=1.0 / float(NS + 1),
                                            scalar2=None, op0=OP.mult)
                    nc.vector.tensor_mul(m_t[:], m_t[:], d_t[:])
                    nc.vector.tensor_add(lo[:], lo[:], m_t[:])
                    nc.vector.tensor_add(hi[:], lo[:], d_t[:])

                lam_t = bis.tile([128, TG, 2], F32)
                nc.vector.tensor_add(lam_t[:], lo[:], hi[:])
                nc.vector.tensor_scalar(out=lam_t[:], in0=lam_t[:],
                                        scalar1=OUT_SCALE, scalar2=None,
                                        op0=OP.mult)
                lam_ap = lam_out.rearrange("(h t p) -> h p t", h=2, t=TG, p=128)
                for h in range(2):
                    nc.sync.dma_start(out=lam_ap[h], in_=lam_t[:, :, h])

    nc.compile()
    return nc


def kernel(x, y, A, B, C, eigval_idx):
    from concourse.bass_utils import run_bass_kernel_spmd

    idx = int(np.asarray(eigval_idx))
    nc = _program(idx)

    # interleave the two batch-halves on even/odd partitions so the
    # ones-block reduction broadcast lands (h0, h1) on adjacent partitions
    perm = np.empty(128, np.int64)
    perm[0::2] = np.arange(64)
    perm[1::2] = 64 + np.arange(64)

    A32 = np.asarray(A, np.float32) * C_OP
    B32 = np.asarray(B, np.float32) * C_OP
    C32 = np.asarray(C, np.float32) * C_OP
    def _pp(m):
        return np.ascontiguousarray(m[perm][:, perm])

    lms = _pp(_bd(A32 + A32.T))
    lbf = _pp(_bd(B32.T))
    lcf = _pp(_bd(C32.T))
    lbt = _pp(_bd(B32))
    lct = _pp(_bd(C32))
    obd = _pp(_bd(np.ones((64, 64), np.float32)))
    eye = np.eye(128, dtype=np.float32)
    idn = -eye
    v0 = np.concatenate([_v0_vec(), _v0_vec()]).reshape(128, 1)[perm]

    xT = np.ascontiguousarray(np.asarray(x, np.float32).T)  # (64, BATCH)
    yT = np.ascontiguousarray(np.asarray(y, np.float32).T)

    in_maps = []
    for c in range(NCORES):
        b0 = c * SHARD
        xc = np.concatenate(
            [xT[:, b0: b0 + NFREE], xT[:, b0 + NFREE: b0 + SHARD]], axis=0
        )
        yc = np.concatenate(
            [yT[:, b0: b0 + NFREE], yT[:, b0 + NFREE: b0 + SHARD]], axis=0
        )
        xc = xc[perm]
        yc = yc[perm]
        xy = np.concatenate(
            [arr for g in range(NCH)
             for arr in (xc[:, g * CHUNK:(g + 1) * CHUNK],
                         yc[:, g * CHUNK:(g + 1) * CHUNK])],
            axis=1,
        )
        in_maps.append(
            {
                "xy": np.ascontiguousarray(xy),
                "lms": lms, "lbf": lbf, "lcf": lcf, "lbt": lbt, "lct": lct,
                "obd": obd, "idp": eye, "idn": idn, "v0": v0,
            }
        )

    res = run_bass_kernel_spmd(nc, in_maps, core_ids=list(range(NCORES)))
    out = np.concatenate([res.results[c]["lam"] for c in range(NCORES)])
    return out.reshape(BATCH, 1).astype(np.float32)



# revision 5
# speedup vs baseline: 1.1717x; 1.1717x over previous
"""Bass/Trainium2 kernel for nn_BivariateSpectral: batched smallest-eigenvalue of
S_b = sym(A + B*diag(x_b) + C*diag(y_b)), b = 0..32767, each 64x64, 8 NeuronCores.

Algorithm (per core, data-parallel over batch):
  Phase 1 - batched Lanczos (K steps) on D_b = (M_b + M_b^T)/64 = S_b/32.
    Batched matvec via shared 128x128 block-diag matmuls (dim on partitions,
    two batch-halves packed on partition halves, batch on the free dim):
      D v = Ah v + Bh (x*v) + Ch (y*v) + x*(Bh^T v) + y*(Ch^T v)
    Engine split per step: gpsimd does the front Hadamards (t1,t2,t4); PE does
    the 5 matvec passes, identity-matmul PSUM accumulation of the post-scaled
    transpose terms and the orthogonalization terms, and the ones-block-diag
    reduction broadcasts (alpha, beta^2); DVE does the remaining Hadamards
    (m12, p_t, t3) and the normalization divide; ACT does Square (beta^2 prep)
    and Sqrt (beta) from one activation table.  Tridiagonal rows (alpha,
    beta^2) are DMA'd straight from PSUM into SBUF staging tiles.
  Phase 2 - divide-form Sturm bisection (count of negative LDL^T pivots) on
    the K x K tridiagonals, batch on partitions, NS shifts/pass, arithmetic
    bracket update (no predicated copies).  Output scaled back by 32.
"""

import functools
import numpy as np

BATCH, DIM = 32768, 64
NCORES = 8
SHARD = BATCH // NCORES      # 4096 batch elems per core
NFREE = SHARD // 2           # 2048 free columns (two partition-halves)
CHUNK = 512                  # columns per group
NCH = NFREE // CHUNK         # 2 groups
K = 23                       # Lanczos steps
NPOW = 5                     # power-iteration boost steps for v0 (v0 <- D^NPOW v0)
ROWS_A = 2 * K               # 64 rows in alpha staging (2j+h)
ROWS_B = 2 * (K - 1)         # 62 rows in beta^2 staging
TG = NFREE // 128            # 16 transpose column-groups
NS = 8                       # bisection shifts per pass
PASSES = 3
C_OP = np.float32(1.0 / 64.0)   # A,B,C host prescale: D = (M+M^T)/64 = S/32
OUT_SCALE = 16.0                # lam_S = 32 * 0.5 * (lo+hi)


def _v0_vec():
    rng = np.random.default_rng(15)
    v = rng.standard_normal(DIM).astype(np.float64)
    v /= np.sqrt((v * v).sum())
    return v.astype(np.float32)


def _bd(m):
    """128x128 block-diagonal duplication of a 64x64 matrix."""
    out = np.zeros((128, 128), np.float32)
    out[:64, :64] = m
    out[64:, 64:] = m
    return out


def _bcast_s(ap, extra_off=0, count=2, ns=NS):
    """Insert a 0-step 'shift' dim after the partition dim of a [128, T, R] AP,
    slicing 'count' elems at free offset extra_off: -> [128, ns, T, count]."""
    import concourse.bass as bass
    dims = list(ap.ap)
    part = dims[0]
    tdim = dims[1]
    return bass.AP(
        tensor=ap.tensor,
        offset=ap.offset + extra_off,
        ap=[part, [0, ns], tdim, [1, count]],
    )


def _bcast_flat(ap, ns=NS):
    """[128, T, 2] AP -> [128, ns, T, 2] via 0-step shift dim."""
    import concourse.bass as bass
    dims = list(ap.ap)
    return bass.AP(tensor=ap.tensor, offset=ap.offset, ap=[dims[0], [0, ns]] + dims[1:])


def _rows2(ap, row0=0, row_step=64, cols=CHUNK):
    """AP for rows {row0, row0+row_step} x [0, cols) of a [128, C] tile."""
    import concourse.bass as bass
    dims = list(ap.ap)
    part = dims[0]
    pstride = part[0]
    return bass.AP(
        tensor=ap.tensor,
        offset=ap.offset,
        ap=[[pstride * row_step, 2], [1, cols]],
    )


def _perm_ns_last(ap, ns=NS, tg=TG):
    """[128, NS, TG, 2] AP -> [128, TG, 2, NS] (reduce target)."""
    import concourse.bass as bass
    dims = list(ap.ap)
    part, nsd, tgd, hd = dims
    return bass.AP(tensor=ap.tensor, offset=ap.offset, ap=[part, tgd, hd, nsd])


@functools.lru_cache(maxsize=4)
def _program(idx: int):
    import concourse.bacc as bacc
    import concourse.bass as bass
    import concourse.mybir as mybir
    import concourse.tile as tile
    from concourse.masks import make_identity

    F32 = mybir.dt.float32
    F32R = mybir.dt.float32r
    OP = mybir.AluOpType
    ACTF = mybir.ActivationFunctionType

    nc = bacc.Bacc("TRN2", target_bir_lowering=False, debug=False)

    xy_in = nc.dram_tensor("xy", [128, 2 * NFREE], F32, kind="ExternalInput").ap()
    lms_in = nc.dram_tensor("lms", [128, 128], F32, kind="ExternalInput").ap()
    lbf_in = nc.dram_tensor("lbf", [128, 128], F32, kind="ExternalInput").ap()
    lcf_in = nc.dram_tensor("lcf", [128, 128], F32, kind="ExternalInput").ap()
    lbt_in = nc.dram_tensor("lbt", [128, 128], F32, kind="ExternalInput").ap()
    lct_in = nc.dram_tensor("lct", [128, 128], F32, kind="ExternalInput").ap()
    obd_in = nc.dram_tensor("obd", [128, 128], F32, kind="ExternalInput").ap()
    idp_in = nc.dram_tensor("idp", [128, 128], F32, kind="ExternalInput").ap()
    idn_in = nc.dram_tensor("idn", [128, 128], F32, kind="ExternalInput").ap()
    v0_in = nc.dram_tensor("v0", [128, 1], F32, kind="ExternalInput").ap()
    lam_out = nc.dram_tensor("lam", [SHARD], F32, kind="ExternalOutput").ap()

    with tile.TileContext(nc) as tc:
        with tc.tile_pool(name="stage", bufs=1) as stg:
            ta_sb = stg.tile([ROWS_A, NFREE], F32)
            tb_sb = stg.tile([ROWS_B, NFREE], F32)

            # ---------------- Phase 1: Lanczos ----------------
            with (
                tc.tile_pool(name="singles", bufs=1) as singles,
                tc.tile_pool(name="vpool", bufs=3) as vpool,
                tc.tile_pool(name="bbp", bufs=2) as bbp,
                tc.tile_pool(name="wk1", bufs=2) as wk1,
                tc.tile_pool(name="wk2", bufs=1) as wk2,
                tc.tile_pool(name="prp", bufs=4) as prp,
                tc.tile_pool(name="ppw", bufs=1, space="PSUM") as ppw,
                tc.tile_pool(name="pps", bufs=1, space="PSUM") as pps,
            ):
                xyt = singles.tile([128, 2 * NFREE], F32)
                nc.sync.dma_start(out=xyt[:], in_=xy_in)
                stat_f32 = {}
                stat_r = {}
                for nm, src in (("lms", lms_in), ("lbf", lbf_in), ("lcf", lcf_in),
                                ("lbt", lbt_in), ("lct", lct_in), ("obd", obd_in),
                                ("idp", idp_in), ("idn", idn_in)):
                    t_f = singles.tile([128, 128], F32, tag=f"stf_{nm}",
                                       name=f"stf_{nm}")
                    nc.sync.dma_start(out=t_f[:], in_=src)
                    t_r = singles.tile([128, 128], F32R, tag=f"str_{nm}",
                                       name=f"str_{nm}")
                    nc.vector.tensor_copy(t_r[:], t_f[:])
                    stat_f32[nm] = t_f
                    stat_r[nm] = t_r
                lms_r, lbf_r, lcf_r = stat_r["lms"], stat_r["lbf"], stat_r["lcf"]
                lbt_r, lct_r, obd_r = stat_r["lbt"], stat_r["lct"], stat_r["obd"]
                idp_r, idn_r = stat_r["idp"], stat_r["idn"]
                v0t = singles.tile([128, 1], F32)
                nc.sync.dma_start(out=v0t[:], in_=v0_in)
                epst = singles.tile([128, 1], F32)
                nc.vector.memset(epst[:], 1e-12)

                st = []
                for g in range(NCH):
                    v_cur = vpool.tile([128, CHUNK], F32R, tag=f"v{g}")
                    nc.vector.tensor_copy(v_cur[:],
                                          v0t[:, 0:1].to_broadcast((128, CHUNK)))
                    st.append({"v": v_cur, "vp": None, "bb": None})

                # ---- power boost: v <- D^NPOW v0 (unnormalized), then normalize
                for it in range(NPOW):
                    PT = [{} for _ in range(NCH)]
                    for g in range(NCH):
                        D = PT[g]
                        xg = xyt[:, g * 2 * CHUNK: g * 2 * CHUNK + CHUNK]
                        yg = xyt[:, g * 2 * CHUNK + CHUNK: (g + 1) * 2 * CHUNK]
                        D["t1"] = wk1.tile([128, CHUNK], F32R, tag=f"t1{g}", name=f"pw_t1{g}")
                        D["t2"] = wk1.tile([128, CHUNK], F32R, tag=f"t2{g}", name=f"pw_t2{g}")
                        nc.gpsimd.tensor_mul(D["t1"][:], xg, st[g]["v"][:])
                        nc.vector.tensor_mul(D["t2"][:], yg, st[g]["v"][:])
                    for g in range(NCH):
                        PT[g]["p3"] = pps.tile([128, CHUNK], F32, tag=f"ps{g}", name=f"pw_p3{g}")
                        PT[g]["pw"] = ppw.tile([128, CHUNK], F32, tag=f"pw{g}", name=f"pw_pw{g}")
                        nc.tensor.matmul(PT[g]["p3"][:], lbt_r[:], st[g]["v"][:],
                                         start=True, stop=True)
                    for g in range(NCH):
                        D = PT[g]
                        D["m1"] = wk2.tile([128, CHUNK], F32R, tag=f"m1{g}", name=f"pw_m1{g}")
                        xg = xyt[:, g * 2 * CHUNK: g * 2 * CHUNK + CHUNK]
                        nc.vector.tensor_mul(D["m1"][:], xg, D["p3"][:])
                    for g in range(NCH):
                        nc.tensor.matmul(PT[g]["pw"][:], lms_r[:], st[g]["v"][:],
                                         start=True, stop=False)
                    for g in range(NCH):
                        nc.tensor.matmul(PT[g]["pw"][:], lbf_r[:], PT[g]["t1"][:],
                                         start=False, stop=False)
                    for g in range(NCH):
                        PT[g]["p4"] = pps.tile([128, CHUNK], F32, tag=f"ps{g}", name=f"pw_p4{g}")
                        nc.tensor.matmul(PT[g]["p4"][:], lct_r[:], st[g]["v"][:],
                                         start=True, stop=True)
                    for g in range(NCH):
                        nc.tensor.matmul(PT[g]["pw"][:], lcf_r[:], PT[g]["t2"][:],
                                         start=False, stop=False)
                    for g in range(NCH):
                        D = PT[g]
                        D["m2"] = wk2.tile([128, CHUNK], F32R, tag=f"m2{g}", name=f"pw_m2{g}")
                        yg = xyt[:, g * 2 * CHUNK + CHUNK: (g + 1) * 2 * CHUNK]
                        nc.vector.tensor_mul(D["m2"][:], yg, D["p4"][:])
                    for g in range(NCH):
                        nc.tensor.matmul(PT[g]["pw"][:], idp_r[:], PT[g]["m1"][:],
                                         start=False, stop=False)
                    for g in range(NCH):
                        nc.tensor.matmul(PT[g]["pw"][:], idp_r[:], PT[g]["m2"][:],
                                         start=False, stop=True)
                    for g in range(NCH):
                        v_nxt = vpool.tile([128, CHUNK], F32R, tag=f"v{g}")
                        nc.scalar.activation(v_nxt[:], PT[g]["pw"][:], ACTF.Copy)
                        st[g]["v"] = v_nxt
                # normalize the boosted v
                NT = [{} for _ in range(NCH)]
                for g in range(NCH):
                    D = NT[g]
                    D["sq"] = wk2.tile([128, CHUNK], F32R, tag=f"q{g}", name=f"nz_sq{g}")
                    nc.scalar.activation(D["sq"][:], st[g]["v"][:], ACTF.Square)
                for g in range(NCH):
                    NT[g]["bc"] = pps.tile([128, CHUNK], F32, tag=f"ps{g}", name=f"nz_bc{g}")
                    nc.tensor.matmul(NT[g]["bc"][:], obd_r[:], NT[g]["sq"][:],
                                     start=True, stop=True)
                for g in range(NCH):
                    D = NT[g]
                    D["nrm"] = bbp.tile([128, CHUNK], F32, tag=f"bb{g}", name=f"nz_nrm{g}")
                    nc.scalar.activation(D["nrm"][:], D["bc"][:], ACTF.Sqrt,
                                         bias=epst[:], scale=1.0)
                    D["rb"] = wk2.tile([128, CHUNK], F32, tag=f"rb{g}", name=f"nz_rb{g}")
                    nc.vector.reciprocal_approx_fast(out=D["rb"][:], in_=D["nrm"][:])
                    v_nxt = vpool.tile([128, CHUNK], F32R, tag=f"v{g}")
                    nc.gpsimd.tensor_mul(v_nxt[:], st[g]["v"][:], D["rb"][:])
                    st[g]["v"] = v_nxt

                for j in range(K):
                    last = j == K - 1
                    T = [{} for _ in range(NCH)]
                    # ---- gpsimd front Hadamards ----
                    for g in range(NCH):
                        S, D = st[g], T[g]
                        xg = xyt[:, g * 2 * CHUNK: g * 2 * CHUNK + CHUNK]
                        yg = xyt[:, g * 2 * CHUNK + CHUNK: (g + 1) * 2 * CHUNK]
                        D["t1"] = wk1.tile([128, CHUNK], F32R, tag=f"t1{g}", name=f"t1{g}")
                        D["t2"] = wk1.tile([128, CHUNK], F32R, tag=f"t2{g}", name=f"t2{g}")
                        nc.gpsimd.tensor_mul(D["t1"][:], xg, S["v"][:])
                        nc.vector.tensor_mul(D["t2"][:], yg, S["v"][:])
                        if j > 0 and not last:
                            D["t4"] = wk2.tile([128, CHUNK], F32R, tag=f"t4{g}", name=f"t4{g}")
                            nc.gpsimd.tensor_mul(D["t4"][:], S["bb"][:], S["vp"][:])
                    # ---- PE matvec (stationary-grouped for LDWEIGHTS reuse) ----
                    for g in range(NCH):
                        T[g]["p3"] = pps.tile([128, CHUNK], F32, tag=f"ps{g}", name=f"p3{g}")
                        T[g]["pw"] = ppw.tile([128, CHUNK], F32, tag=f"pw{g}", name=f"pw{g}")
                    for g in range(NCH):
                        for n0 in range(0, CHUNK, 512):
                            ns = slice(n0, n0 + 512)
                            nc.tensor.matmul(T[g]["p3"][:, ns], lbt_r[:],
                                             st[g]["v"][:, ns], start=True, stop=True)
                    # m1 = x * (Bh^T v) frees the p3 slot for p4
                    for g in range(NCH):
                        D = T[g]
                        D["m1"] = wk2.tile([128, CHUNK], F32R, tag=f"m1{g}", name=f"m1{g}")
                        xg = xyt[:, g * 2 * CHUNK: g * 2 * CHUNK + CHUNK]
                        nc.vector.tensor_mul(D["m1"][:], xg, D["p3"][:])
                    # pw passes (independent) hide the p3->p4 slot turnaround
                    for g in range(NCH):
                        for n0 in range(0, CHUNK, 512):
                            ns = slice(n0, n0 + 512)
                            nc.tensor.matmul(T[g]["pw"][:, ns], lms_r[:],
                                             st[g]["v"][:, ns], start=True, stop=False)
                    for g in range(NCH):
                        for n0 in range(0, CHUNK, 512):
                            ns = slice(n0, n0 + 512)
                            nc.tensor.matmul(T[g]["pw"][:, ns], lbf_r[:],
                                             T[g]["t1"][:, ns], start=False, stop=False)
                    for g in range(NCH):
                        T[g]["p4"] = pps.tile([128, CHUNK], F32, tag=f"ps{g}", name=f"p4{g}")
                        for n0 in range(0, CHUNK, 512):
                            ns = slice(n0, n0 + 512)
                            nc.tensor.matmul(T[g]["p4"][:, ns], lct_r[:],
                                             st[g]["v"][:, ns], start=True, stop=True)
                    for g in range(NCH):
                        for n0 in range(0, CHUNK, 512):
                            ns = slice(n0, n0 + 512)
                            nc.tensor.matmul(T[g]["pw"][:, ns], lcf_r[:],
                                             T[g]["t2"][:, ns], start=False, stop=False)
                    for g in range(NCH):
                        D = T[g]
                        D["m2"] = wk2.tile([128, CHUNK], F32R, tag=f"m2{g}", name=f"m2{g}")
                        yg = xyt[:, g * 2 * CHUNK + CHUNK: (g + 1) * 2 * CHUNK]
                        nc.vector.tensor_mul(D["m2"][:], yg, D["p4"][:])
                    # ---- PE identity accumulate of m1, m2 into pw ----
                    for g in range(NCH):
                        for n0 in range(0, CHUNK, 512):
                            ns = slice(n0, n0 + 512)
                            nc.tensor.matmul(T[g]["pw"][:, ns], idp_r[:],
                                             T[g]["m1"][:, ns], start=False,
                                             stop=False)
                    for g in range(NCH):
                        for n0 in range(0, CHUNK, 512):
                            ns = slice(n0, n0 + 512)
                            nc.tensor.matmul(T[g]["pw"][:, ns], idp_r[:],
                                             T[g]["m2"][:, ns], start=False,
                                             stop=True)
                    # ---- DVE p_t = v * w ----
                    for g in range(NCH):
                        D = T[g]
                        D["pt"] = wk2.tile([128, CHUNK], F32R, tag=f"pt{g}", name=f"ptl{g}")
                        nc.vector.tensor_mul(D["pt"][:], st[g]["v"][:], D["pw"][:])
                    # ---- PE alpha broadcast ----
                    for g in range(NCH):
                        T[g]["bc"] = pps.tile([128, CHUNK], F32, tag=f"ps{g}", name=f"bc{g}")
                        for n0 in range(0, CHUNK, 512):
                            ns = slice(n0, n0 + 512)
                            nc.tensor.matmul(T[g]["bc"][:, ns], obd_r[:],
                                             T[g]["pt"][:, ns], start=True, stop=True)
                    # ---- stage alpha rows: ACT bounce (aligned) + DMA to row 2j ----
                    for g in range(NCH):
                        csl = slice(g * CHUNK, (g + 1) * CHUNK)
                        pr = prp.tile([2, CHUNK], F32, tag="pr", name=f"pr{g}")
                        nc.scalar.activation(pr[:], T[g]["bc"][0:2, 0:CHUNK],
                                             ACTF.Copy)
                        nc.sync.dma_start(out=ta_sb[2 * j: 2 * j + 2, csl],
                                          in_=pr[:])
                    if last:
                        continue
                    # ---- DVE t3 = alpha * v ----
                    for g in range(NCH):
                        D = T[g]
                        D["t3"] = wk2.tile([128, CHUNK], F32R, tag=f"t3{g}", name=f"t3{g}")
                        nc.vector.tensor_mul(D["t3"][:], D["bc"][:], st[g]["v"][:])
                    # ---- PE -I accumulate of t3 (and t4) into pw ----
                    for g in range(NCH):
                        for n0 in range(0, CHUNK, 512):
                            ns = slice(n0, n0 + 512)
                            nc.tensor.matmul(T[g]["pw"][:, ns], idn_r[:],
                                             T[g]["t3"][:, ns], start=False,
                                             stop=(j == 0), skip_group_check=True)
                    if j > 0:
                        for g in range(NCH):
                            for n0 in range(0, CHUNK, 512):
                                ns = slice(n0, n0 + 512)
                                nc.tensor.matmul(T[g]["pw"][:, ns], idn_r[:],
                                                 T[g]["t4"][:, ns], start=False,
                                                 stop=True, skip_group_check=True)
                    # ---- ACT ws = copy(w_hat) to SBUF; q = Square(ws) ----
                    for g in range(NCH):
                        D = T[g]
                        D["ws"] = wk2.tile([128, CHUNK], F32R, tag=f"ws{g}", name=f"ws{g}")
                        nc.scalar.activation(D["ws"][:], D["pw"][:], ACTF.Copy)
                    for g in range(NCH):
                        D = T[g]
                        D["q"] = wk2.tile([128, CHUNK], F32R, tag=f"q{g}", name=f"q{g}")
                        nc.scalar.activation(D["q"][:], D["ws"][:], ACTF.Square)
                    # ---- PE beta^2 broadcast (reuse bc tag -> WAR dep) ----
                    for g in range(NCH):
                        T[g]["bc2"] = pps.tile([128, CHUNK], F32, tag=f"ps{g}", name=f"bc2{g}")
                        for n0 in range(0, CHUNK, 512):
                            ns = slice(n0, n0 + 512)
                            nc.tensor.matmul(T[g]["bc2"][:, ns], obd_r[:],
                                             T[g]["q"][:, ns], start=True, stop=True)
                    # ---- ACT bb = Sqrt(beta^2 + eps); v_nxt = ws * (1/bb) ----
                    # beta rows staged straight from bb (SBUF) via DMA;
                    # phase 2 squares them back to beta^2.
                    for g in range(NCH):
                        D = T[g]
                        bbn = bbp.tile([128, CHUNK], F32, tag=f"bb{g}")
                        nc.scalar.activation(bbn[:], D["bc2"][:], ACTF.Sqrt,
                                             bias=epst[:], scale=1.0)
                        csl = slice(g * CHUNK, (g + 1) * CHUNK)
                        nc.sync.dma_start(out=tb_sb[2 * j: 2 * j + 2, csl],
                                          in_=bbn[0:2, 0:CHUNK])
                        rb = wk2.tile([128, CHUNK], F32, tag=f"rb{g}", name=f"rb{g}")
                        nc.vector.reciprocal_approx_fast(out=rb[:], in_=bbn[:])
                        v_nxt = vpool.tile([128, CHUNK], F32R, tag=f"v{g}")
                        nc.gpsimd.tensor_mul(v_nxt[:], D["ws"][:], rb[:])
                        st[g]["vp"] = st[g]["v"]
                        st[g]["v"] = v_nxt
                        st[g]["bb"] = bbn

            # ---------------- Phase 2: transpose + Sturm bisection ----------------
            with (
                tc.tile_pool(name="bis", bufs=1) as bis,
                tc.tile_pool(name="st3", bufs=1) as st3,
                tc.tile_pool(name="pt", bufs=2, space="PSUM") as ptp,
            ):
                ident = bis.tile([128, 128], F32)
                make_identity(nc, ident[:])

                td_a = bis.tile([128, TG, ROWS_A], F32)
                td_b = bis.tile([128, TG, ROWS_B], F32)
                for t in range(TG):
                    csl = slice(t * 128, (t + 1) * 128)
                    pa = ptp.tile([128, ROWS_A], F32, tag="pt")
                    nc.tensor.transpose(pa[:], ta_sb[:, csl],
                                        ident[0:ROWS_A, 0:ROWS_A])
                    nc.vector.tensor_copy(td_a[:, t, :], pa[:])
                    pb = ptp.tile([128, ROWS_B], F32, tag="pt")
                    nc.tensor.transpose(pb[:], tb_sb[:, csl],
                                        ident[0:ROWS_B, 0:ROWS_B])
                    nc.vector.tensor_copy(td_b[:, t, :], pb[:])


                import concourse.bass as bass_mod

                def jdims_ap(tile_ap, nj, step0=2):
                    d = list(tile_ap.ap)
                    return bass_mod.AP(
                        tensor=tile_ap.tensor, offset=tile_ap.offset,
                        ap=[d[0], d[1], [1, 2], [step0, nj]],
                    )

                # td_b holds |beta_j| (= sqrt(beta^2+eps)) as staged
                absb = td_b
                g_t = bis.tile([128, TG, ROWS_A], F32)
                nc.vector.tensor_copy(g_t[:], td_a[:])
                nc.vector.tensor_sub(g_t[:, :, 2:ROWS_A], g_t[:, :, 2:ROWS_A], absb[:])
                nc.vector.tensor_sub(g_t[:, :, 0:ROWS_B], g_t[:, :, 0:ROWS_B], absb[:])

                lo = bis.tile([128, TG, 2], F32)
                hi = bis.tile([128, TG, 2], F32)
                nc.vector.tensor_reduce(lo[:], jdims_ap(g_t[:], K),
                                        mybir.AxisListType.X, OP.min)
                if idx == 0:
                    nc.vector.tensor_reduce(hi[:], jdims_ap(td_a[:], K),
                                            mybir.AxisListType.X, OP.min)
                else:
                    g2 = g_t
                    nc.vector.tensor_copy(g2[:], td_a[:])
                    nc.vector.tensor_add(g2[:, :, 2:ROWS_A], g2[:, :, 2:ROWS_A],
                                         absb[:])
                    nc.vector.tensor_add(g2[:, :, 0:ROWS_B], g2[:, :, 0:ROWS_B],
                                         absb[:])
                    nc.vector.tensor_reduce(hi[:], jdims_ap(g2[:], K),
                                            mybir.AxisListType.X, OP.max)

                # square |beta| back to beta^2 for the Sturm recurrence
                nc.vector.tensor_mul(td_b[:], td_b[:], td_b[:])

                cs = bis.tile([128, NS, TG, 2], F32)
                for s in range(NS):
                    nc.vector.memset(cs[:, s, :, :], float(s + 1) / float(NS + 1))

                sig = st3.tile([128, NS, TG, 2], F32, tag="sig")
                pA = st3.tile([128, NS, TG, 2], F32, tag="pA")
                pB = st3.tile([128, NS, TG, 2], F32, tag="pB")
                pC = st3.tile([128, NS, TG, 2], F32, tag="pC")
                ca_t = st3.tile([128, NS, TG, 2], F32, tag="ca")
                u_t = st3.tile([128, NS, TG, 2], F32, tag="u")
                tb_t = st3.tile([128, NS, TG, 2], F32, tag="tb")
                sg_t = st3.tile([128, NS, TG, 2], F32, tag="sg")
                cA = st3.tile([128, NS, TG, 2], F32, tag="cA")
                cB = st3.tile([128, NS, TG, 2], F32, tag="cB")
                les = st3.tile([128, NS, TG, 2], F32, tag="les")
                d_t = bis.tile([128, TG, 2], F32)
                m_t = bis.tile([128, TG, 2], F32)

                thr = float(idx) + 0.5
                for ip in range(PASSES):
                    nc.vector.tensor_sub(d_t[:], hi[:], lo[:])
                    nc.vector.tensor_mul(sig[:], cs[:], _bcast_flat(d_t[:]))
                    nc.vector.tensor_add(sig[:], sig[:], _bcast_flat(lo[:]))
                    po, pp, pn = pA, pB, pC
                    nc.vector.memset(po[:], 1.0)
                    nc.vector.tensor_sub(pp[:], _bcast_s(td_a[:], 0), sig[:])
                    cnt, cnt_nxt = cA, cB
                    nc.vector.tensor_scalar(out=cnt[:], in0=pp[:], scalar1=0.0,
                                            scalar2=None, op0=OP.is_lt)
                    for j in range(1, K):
                        nc.vector.tensor_sub(ca_t[:], _bcast_s(td_a[:], 2 * j),
                                             sig[:])
                        nc.vector.tensor_mul(u_t[:], ca_t[:], pp[:])
                        nc.vector.tensor_mul(tb_t[:], _bcast_s(td_b[:], 2 * (j - 1)),
                                             po[:])
                        nc.vector.tensor_sub(pn[:], u_t[:], tb_t[:])
                        nc.vector.tensor_mul(sg_t[:], pn[:], pp[:])
                        nc.vector.scalar_tensor_tensor(
                            out=cnt_nxt[:], in0=sg_t[:], scalar=0.0, in1=cnt[:],
                            op0=OP.is_lt, op1=OP.add)
                        po, pp, pn = pp, pn, po
                        cnt, cnt_nxt = cnt_nxt, cnt
                    # arithmetic bracket update: m = #shifts with cnt <= idx
                    nc.vector.tensor_scalar(out=les[:], in0=cnt[:], scalar1=thr,
                                            scalar2=None, op0=OP.is_le)
                    nc.vector.tensor_reduce(m_t[:], _perm_ns_last(les[:]),
                                            mybir.AxisListType.X, OP.add)
                    nc.vector.tensor_scalar(out=d_t[:], in0=d_t[:],
                                            scalar1=1.0 / float(NS + 1),
                                            scalar2=None, op0=OP.mult)
                    nc.vector.tensor_mul(m_t[:], m_t[:], d_t[:])
                    nc.vector.tensor_add(lo[:], lo[:], m_t[:])
                    nc.vector.tensor_add(hi[:], lo[:], d_t[:])

                lam_t = bis.tile([128, TG, 2], F32)
                nc.vector.tensor_add(lam_t[:], lo[:], hi[:])
                nc.vector.tensor_scalar(out=lam_t[:], in0=lam_t[:],
                                        scalar1=OUT_SCALE, scalar2=None,
                                        op0=OP.mult)
                lam_ap = lam_out.rearrange("(h t p) -> h p t", h=2, t=TG, p=128)
                for h in range(2):
                    nc.sync.dma_start(out=lam_ap[h], in_=lam_t[:, :, h])

    nc.compile()
    return nc


def kernel(x, y, A, B, C, eigval_idx):
    from concourse.bass_utils import run_bass_kernel_spmd

    idx = int(np.asarray(eigval_idx))
    nc = _program(idx)

    # interleave the two batch-halves on even/odd partitions so the
    # ones-block reduction broadcast lands (h0, h1) on adjacent partitions
    perm = np.empty(128, np.int64)
    perm[0::2] = np.arange(64)
    perm[1::2] = 64 + np.arange(64)

    A32 = np.asarray(A, np.float32) * C_OP
    B32 = np.asarray(B, np.float32) * C_OP
    C32 = np.asarray(C, np.float32) * C_OP
    def _pp(m):
        return np.ascontiguousarray(m[perm][:, perm])

    lms = _pp(_bd(A32 + A32.T))
    lbf = _pp(_bd(B32.T))
    lcf = _pp(_bd(C32.T))
    lbt = _pp(_bd(B32))
    lct = _pp(_bd(C32))
    obd = _pp(_bd(np.ones((64, 64), np.float32)))
    eye = np.eye(128, dtype=np.float32)
    idn = -eye
    v0 = np.concatenate([_v0_vec(), _v0_vec()]).reshape(128, 1)[perm]

    xT = np.ascontiguousarray(np.asarray(x, np.float32).T)  # (64, BATCH)
    yT = np.ascontiguousarray(np.asarray(y, np.float32).T)

    in_maps = []
    for c in range(NCORES):
        b0 = c * SHARD
        xc = np.concatenate(
            [xT[:, b0: b0 + NFREE], xT[:, b0 + NFREE: b0 + SHARD]], axis=0
        )
        yc = np.concatenate(
            [yT[:, b0: b0 + NFREE], yT[:, b0 + NFREE: b0 + SHARD]], axis=0
        )
        xc = xc[perm]
        yc = yc[perm]
        xy = np.concatenate(
            [arr for g in range(NCH)
             for arr in (xc[:, g * CHUNK:(g + 1) * CHUNK],
                         yc[:, g * CHUNK:(g + 1) * CHUNK])],
            axis=1,
        )
        in_maps.append(
            {
                "xy": np.ascontiguousarray(xy),
                "lms": lms, "lbf": lbf, "lcf": lcf, "lbt": lbt, "lct": lct,
                "obd": obd, "idp": eye, "idn": idn, "v0": v0,
            }
        )

    res = run_bass_kernel_spmd(nc, in_maps, core_ids=list(range(NCORES)))
    out = np.concatenate([res.results[c]["lam"] for c in range(NCORES)])
    return out.reshape(BATCH, 1).astype(np.float32)

